# revision 1
# baseline (speedup 1.0000x reference)
"""Trainium2 Bass kernel for a BFP-quantized ResNet BasicBlock (inference).

Computes, per image (NCHW, C=128, H=W=56):
    out = relu( bn2( conv3x3( q( relu(bn1( conv3x3(q(x), q(w1)) )) ), q(w2)) ) + x )
where q() is HBFP block-floating-point quantization: blocks of 64 contiguous
values (in flat row-major order) share a power-of-2 scale 2^(floor(log2(max|x|))-7),
mantissas rounded (RNE) to 8 signed bits and clamped to +-127.

Key facts exploited:
  * Quantized values are (int in [-127,127]) * 2^k  -> exactly representable in
    bf16, so convs run on the PE at bf16 speed with zero extra error.
  * floor(log2(m)) for normal floats == exponent-field extraction (bitwise ops).
  * RNE rounding == (t + 1.5*2^23) - 1.5*2^23 in fp32 (one dual-op tensor_scalar).
  * clip(round(t)) == round(clamp(t, -127.4, 127.4)) elementwise.
  * conv3x3 = 9 accumulated matmuls (C_in=128 on partitions) over a zero-padded
    58-pitch image layout, fully contiguous rhs slices of 464 columns (8 rows).

Sharding: data-parallel over batch N=64 -> 8 images per NeuronCore, weights and
BN constants replicated. All 8 cores run the same NEFF (SPMD).
"""

import os

os.environ.setdefault("MYCRO_LOCAL_CACHE", "1")

from contextlib import ExitStack
from functools import lru_cache

import numpy as np

import concourse.bass as bass
import concourse.tile as tile
from concourse import bacc, mybir
from concourse.bass_utils import run_bass_kernel_spmd
from concourse.masks import make_identity

P = 128
H = W = 56
HWF = H * W            # 3136 flat pixels per channel
NBX = HWF // 64        # 49 BFP blocks per channel image
WLEN = 128 * 9         # 1152 flat weight row per output channel
NBW = WLEN // 64       # 18 BFP blocks per weight row
PITCH = W + 2          # 58 padded row pitch
PADLEN = PITCH * PITCH + 2  # 3366: [1 pre-pad][58x58 padded image][1 post-pad]
NCHUNK = 7             # 8-row output chunks per image
CHF = 8 * PITCH        # 464 matmul free dim per chunk
CROUND = 12582912.0    # 1.5 * 2**23  (RNE magic constant)
CLIPV = 127.4
EXPMASK = 0x7F800000
BIAS7 = 7 << 23
C254 = 254 << 23
EGUARD = 50 << 23      # exponent field of 1e-23 (reference's zero-guard)
BN_EPS = 1e-5

F32 = mybir.dt.float32
BF16 = mybir.dt.bfloat16
I32 = mybir.dt.int32
ALU = mybir.AluOpType
ACTF = mybir.ActivationFunctionType
AX = mybir.AxisListType

N_CORES = 8
NIMG = 8  # images per core


def _emit_quant(nc, small, tmp_pool, src_ap, dst_tile, nb, halves=None,
                clip_gpsimd=True):
    """BFP-quantize src_ap (f32 [P, nb*64]) into dst_tile (bf16 [P, nb*64]).

    halves: optional list of (block_start, block_count) spans; the elementwise
    chain is emitted per span so downstream consumers can start before the
    whole tensor is produced (used for quant2, whose input streams in chunks).
    Engine split: reduce/mul/round on VectorE, clip on GpSimd.
    """
    src3 = src_ap.rearrange("p (b e) -> p b e", e=64)
    dst3 = dst_tile[:].rearrange("p (b e) -> p b e", e=64)
    if halves is None:
        halves = [(0, nb)]

    bm = small.tile([P, nb], F32, tag=f"bm{nb}")
    sb = small.tile([P, nb], I32, tag=f"sb{nb}")
    rb = small.tile([P, nb], I32, tag=f"rb{nb}")
    sc_bf = small.tile([P, nb], BF16, tag=f"scbf{nb}")
    t = tmp_pool.tile([P, nb * 64], F32, tag="qtmp")
    t3 = t[:].rearrange("p (b e) -> p b e", e=64)
    clip_eng = nc.gpsimd if clip_gpsimd else nc.vector
    for b0, bn in halves:
        sl = slice(b0, b0 + bn)
        nc.vector.tensor_reduce(
            out=bm[:, sl], in_=src3[:, sl], axis=AX.X,
            op=ALU.max, apply_absolute_value=True,
        )
        # scale bits = max(exponent field, expfield(1e-23)) - (7 << 23)
        # (the max reproduces the reference's +1e-23 zero-guard)
        nc.vector.tensor_scalar(sb[:, sl], bm[:, sl].bitcast(I32), EXPMASK,
                                None, ALU.bitwise_and)
        nc.vector.tensor_scalar(sb[:, sl], sb[:, sl], EGUARD, BIAS7,
                                ALU.max, ALU.subtract)
        # rscale bits = (254 << 23) - scale_bits  -> rscale = 2^(7-e) = 1/scale
        nc.vector.tensor_scalar(rb[:, sl], sb[:, sl], C254, -1,
                                ALU.subtract, ALU.mult)
        nc.vector.tensor_copy(sc_bf[:, sl], sb[:, sl].bitcast(F32))
        rsc = rb[:].bitcast(F32)[:, sl, None].to_broadcast((P, bn, 64))
        nc.vector.tensor_tensor(t3[:, sl], src3[:, sl], rsc, ALU.mult)
        clip_eng.tensor_scalar(t3[:, sl], t3[:, sl], CLIPV, -CLIPV, ALU.min, ALU.max)
        # RNE round; result is a small integer -> exact in bf16
        nc.vector.tensor_scalar(dst3[:, sl], t3[:, sl], CROUND, CROUND,
                                ALU.add, ALU.subtract)
        scb = sc_bf[:][:, sl, None].to_broadcast((P, bn, 64))
        nc.gpsimd.tensor_tensor(dst3[:, sl], dst3[:, sl], scb, ALU.mult)


def _interior(pad_tile):
    """[P, 56, 56] strided view (pitch 58) of the padded tile's interior."""
    base = 1 + PITCH + 1  # (h=0, w=0) -> index 1 + (h+1)*58 + (w+1)
    v = pad_tile[:, base : base + H * PITCH]
    return v.rearrange("p (h w) -> p h w", w=PITCH)[:, :, :W]


def _emit_conv(nc, psum_pool, wk, src_pad, evict):
    """3x3 conv via 9 accumulated matmuls per 8-row chunk; evict(c, psum)."""
    for c in range(NCHUNK):
        h0 = c * 8
        ps = psum_pool.tile([P, CHF], F32, tag="pschunk")
        for k in range(9):
            kh, kw = divmod(k, 3)
            s = (h0 + kh) * PITCH + kw
            nc.tensor.matmul(
                ps[:], wk[k][:], src_pad[:, s : s + CHF],
                start=(k == 0), stop=(k == 8),
            )
        evict(c, ps)


def build_nc(nimg=NIMG):
    nc = bacc.Bacc("TRN2", target_bir_lowering=False, debug=False,
                   enable_asserts=False)

    x_d = nc.dram_tensor("x", [nimg, P, H, W], F32, kind="ExternalInput").ap()
    w1_d = nc.dram_tensor("w1", [P, P, 3, 3], F32, kind="ExternalInput").ap()
    w2_d = nc.dram_tensor("w2", [P, P, 3, 3], F32, kind="ExternalInput").ap()
    bn_d = {
        name: nc.dram_tensor(name, [P], F32, kind="ExternalInput").ap()
        for name in ("gamma1", "beta1", "mean1", "var1",
                     "gamma2", "beta2", "mean2", "var2")
    }
    out_d = nc.dram_tensor("out", [nimg, P, H, W], F32, kind="ExternalOutput").ap()

    with tile.TileContext(nc) as tc, ExitStack() as ctx:
        const = ctx.enter_context(tc.tile_pool(name="const", bufs=1))
        small = ctx.enter_context(tc.tile_pool(name="small", bufs=4))
        tmp = ctx.enter_context(tc.tile_pool(name="tmp", bufs=2))
        pads = ctx.enter_context(tc.tile_pool(name="pads", bufs=1))

        # ---- setup: weights quant + transpose, BN constants ----
        with tc.tile_pool(name="setup", bufs=1) as setup, \
             tc.tile_pool(name="psum_setup", bufs=2, space="PSUM") as psum_setup:
            ident = const.tile([P, P], BF16, tag="ident")
            make_identity(nc, ident[:])
            zero_b = const.tile([P, 1], F32, tag="zero_b")
            nc.vector.memset(zero_b[:], 0.0)
            eps_b = const.tile([P, 1], F32, tag="eps_b")
            nc.vector.memset(eps_b[:], BN_EPS)

            wks = []
            for wi, w_d in enumerate((w1_d, w2_d)):
                wraw = setup.tile([P, WLEN], F32, tag=f"wraw{wi}")
                nc.sync.dma_start(wraw[:], w_d.rearrange("o i kh kw -> o (i kh kw)"))
                wq = setup.tile([P, WLEN], BF16, tag=f"wq{wi}")
                _emit_quant(nc, small, tmp, wraw[:], wq, NBW)
                # per-offset lhsT tiles: w[k][i, o] = wq[o, i*9+k]
                wq_v = wq[:].rearrange("p (i k) -> p k i", k=9)
                wk = []
                for k in range(9):
                    pt = psum_setup.tile([P, P], BF16, tag="tps")
                    nc.tensor.transpose(pt[:], wq_v[:, k, :], ident[:])
                    wt = const.tile([P, P], BF16, tag=f"w{wi}k{k}")
                    nc.scalar.copy(wt[:], pt[:])
                    wk.append(wt)
                wks.append(wk)
            w1k, w2k = wks

            bnc = {}
            for name in ("gamma1", "beta1", "mean1", "var1",
                         "gamma2", "beta2", "mean2", "var2"):
                t = setup.tile([P, 1], F32, tag=f"bn_{name}")
                nc.sync.dma_start(t[:], bn_d[name][:, None])
                bnc[name] = t
            invb = []
            for i in ("1", "2"):
                s = setup.tile([P, 1], F32, tag=f"sd{i}")
                nc.scalar.activation(s[:], bnc[f"var{i}"][:], ACTF.Sqrt, bias=eps_b[:])
                r = setup.tile([P, 1], F32, tag=f"rs{i}")
                nc.vector.reciprocal(r[:], s[:])
                inv = const.tile([P, 1], F32, tag=f"inv{i}")
                nc.vector.tensor_tensor(inv[:], bnc[f"gamma{i}"][:], r[:], ALU.mult)
                mi = setup.tile([P, 1], F32, tag=f"mi{i}")
                nc.vector.tensor_tensor(mi[:], bnc[f"mean{i}"][:], inv[:], ALU.mult)
                b = const.tile([P, 1], F32, tag=f"b{i}")
                nc.vector.tensor_tensor(b[:], bnc[f"beta{i}"][:], mi[:], ALU.subtract)
                invb.append((inv, b))
            (inv1, b1), (inv2, b2) = invb

        xq_pads = [pads.tile([P, PADLEN], BF16, tag=f"xqp{i}", name=f"xqp{i}")
                   for i in range(2)]
        mq_pads = [pads.tile([P, PADLEN], BF16, tag=f"mqp{i}", name=f"mqp{i}")
                   for i in range(2)]
        for t in (*xq_pads, *mq_pads):
            nc.vector.memset(t[:], 0.0)

        xraw_p = ctx.enter_context(tc.tile_pool(name="xraw", bufs=3))
        u_p = ctx.enter_context(tc.tile_pool(name="u", bufs=2))
        mid_p = ctx.enter_context(tc.tile_pool(name="mid", bufs=2))
        t2_p = ctx.enter_context(tc.tile_pool(name="t2", bufs=2))
        psum1_p = ctx.enter_context(tc.tile_pool(name="psum1", bufs=4, space="PSUM"))
        psum2_p = ctx.enter_context(tc.tile_pool(name="psum2", bufs=4, space="PSUM"))

        # Software-pipelined emission ordered by criticality. Per-engine
        # queues execute in (roughly) emission order, so the latency-critical
        # quant2(n) -> conv2(n) chain is emitted FIRST each iteration; slack
        # work (load/quant1 two images ahead, previous image's residual tail)
        # is emitted after so it fills queue gaps instead of blocking.
        xraws = [None] * nimg
        mids = [None] * nimg
        t2s = [None] * nimg

        def load_quant1(n):
            xq_pad = xq_pads[n % 2]
            xraw = xraw_p.tile([P, HWF], F32, tag="xraw", name=f"xraw{n}")
            xraws[n] = xraw
            nc.sync.dma_start(xraw[:], x_d[n].rearrange("c h w -> c (h w)"))
            u = u_p.tile([P, HWF], BF16, tag="u", name=f"u{n}")
            _emit_quant(nc, small, tmp, xraw[:], u, NBX)
            nc.scalar.dma_start(_interior(xq_pad),
                                u[:].rearrange("p (h w) -> p h w", w=W))

        def conv1(n):
            mid = mid_p.tile([P, HWF], F32, tag="mid", name=f"mid{n}")
            mids[n] = mid

            def evict1(c, ps):
                psv = ps[:].rearrange("p (r w) -> p r w", w=PITCH)[:, :, 1 : 1 + W]
                ov = mid[:, c * 448 : (c + 1) * 448].rearrange("p (r w) -> p r w", w=W)
                nc.scalar.activation(ov, psv, ACTF.Relu, bias=b1[:], scale=inv1[:])

            _emit_conv(nc, psum1_p, w1k, xq_pads[n % 2][:], evict1)

        def quant2(n):
            mq_pad = mq_pads[n % 2]
            u2 = u_p.tile([P, HWF], BF16, tag="u2", name=f"u2_{n}")
            # halves aligned to evict1's chunks: first half starts as soon as
            # conv1's first 4 chunks are evicted
            _emit_quant(nc, small, tmp, mids[n][:], u2, NBX,
                        halves=[(0, 28), (28, 21)])
            nc.scalar.dma_start(_interior(mq_pad),
                                u2[:].rearrange("p (h w) -> p h w", w=W))

        def conv2(n):
            t2 = t2_p.tile([P, HWF], F32, tag="t2", name=f"t2_{n}")
            t2s[n] = t2

            def evict2(c, ps):
                psv = ps[:].rearrange("p (r w) -> p r w", w=PITCH)[:, :, 1 : 1 + W]
                ov = t2[:, c * 448 : (c + 1) * 448].rearrange("p (r w) -> p r w", w=W)
                nc.scalar.activation(ov, psv, ACTF.Identity, bias=b2[:], scale=inv2[:])

            _emit_conv(nc, psum2_p, w2k, mq_pads[n % 2][:], evict2)

        def final(n):
            t2 = t2s[n]
            nc.vector.tensor_tensor(t2[:], t2[:], xraws[n][:], ALU.add)
            nc.scalar.activation(t2[:], t2[:], ACTF.Relu, bias=zero_b[:])
            nc.sync.dma_start(out_d[n].rearrange("c h w -> c (h w)"), t2[:])

        load_quant1(0)
        load_quant1(1)
        conv1(0)
        for n in range(nimg):
            quant2(n)
            if n + 1 < nimg:
                conv1(n + 1)
            conv2(n)
            if n + 2 < nimg:
                load_quant1(n + 2)
            if n >= 1:
                final(n - 1)
        final(nimg - 1)

    nc.compile()
    return nc


@lru_cache(maxsize=1)
def _get_nc():
    return build_nc(NIMG)


def kernel(x, w1, w2, gamma1, beta1, mean1, var1,
           gamma2, beta2, mean2, var2, _trace=False):
    f = lambda a: np.ascontiguousarray(np.asarray(a, dtype=np.float32))
    x = f(x)
    n_total = x.shape[0]
    assert n_total == N_CORES * NIMG, x.shape
    xs = x.reshape(N_CORES, NIMG, P, H, W)
    rep = {
        "w1": f(w1), "w2": f(w2),
        "gamma1": f(gamma1), "beta1": f(beta1), "mean1": f(mean1), "var1": f(var1),
        "gamma2": f(gamma2), "beta2": f(beta2), "mean2": f(mean2), "var2": f(var2),
    }
    in_maps = [{"x": np.ascontiguousarray(xs[c]), **rep} for c in range(N_CORES)]
    nc = _get_nc()
    res = run_bass_kernel_spmd(nc, in_maps, core_ids=list(range(N_CORES)),
                               trace=_trace)
    out = np.concatenate([res.results[c]["out"] for c in range(N_CORES)], axis=0)
    if _trace:
        kernel.last_result = res
    return out.reshape(n_total, P, H, W)

